# revision 1
# baseline (speedup 1.0000x reference)
"""Trainium2 8-core kernel for 2-layer GAT (nn_DiGCN_65335042507185).

Strategy: nodes partitioned across 8 cores by dst (12500/core). Per layer the
host materializes a per-core edge stream (pre-gathered source features +
edge-score pre-activations) ordered by (dst-window, tile, slot); the device
does all model compute: edge softmax weights (exp/leaky), windowed one-hot
segmented aggregation on TensorE with z ridden along as an extra column,
normalization, the W matmul, and relu. Two NEFF launches (one per GAT layer);
between them the host assembles h and builds the layer-2 stream.
"""
import sys
for _p in ("/opt/trn_rl_repo", "/root/.axon_site/_ro/trn_rl_repo"):
    if _p not in sys.path:
        sys.path.insert(0, _p)

import numpy as np
import ml_dtypes
from contextlib import ExitStack

import concourse.bass as bass
import concourse.bacc as bacc
import concourse.mybir as mybir
import concourse.tile as tile
from concourse.bass_utils import run_bass_kernel_spmd

P = 128
N = 100_000
E = 1_600_000
NFEAT = 128
NHID = 64
NEG_SLOPE = 0.2
NCORES = 8
NSH = 12500                 # nodes per core
WPC = 104                   # windows per core (13312 padded nodes)
NODES_PAD = WPC * P
TW = 20                     # tiles per window (2560 slots)
SLOTS = WPC * TW * P        # 252928 edge slots per core
AF = mybir.ActivationFunctionType
DT = mybir.dt
BF16 = ml_dtypes.bfloat16

_CACHE = {}


# ---------------------------------------------------------------- device ----

def _build_layer(F_in, F_out, n_win, t_w, relu):
    C = F_in + 4
    R = F_in + 1
    nc = bacc.Bacc("TRN2", target_bir_lowering=False, debug=False,
                   num_devices=NCORES)
    stream = nc.dram_tensor("stream", [n_win, P, t_w * C], DT.bfloat16,
                            kind="ExternalInput").ap()
    w_hbm = nc.dram_tensor("w", [F_in, F_out], DT.float32,
                           kind="ExternalInput").ap()
    ident_hbm = nc.dram_tensor("ident", [P, P], DT.bfloat16,
                               kind="ExternalInput").ap()
    iota_hbm = nc.dram_tensor("iota", [P, P], DT.bfloat16,
                              kind="ExternalInput").ap()
    outT = nc.dram_tensor("outT", [F_out, n_win * P], DT.float32,
                          kind="ExternalOutput").ap()

    with tile.TileContext(nc) as tc, ExitStack() as ctx:
        cpool = ctx.enter_context(tc.tile_pool(name="consts", bufs=1))
        w_sb = cpool.tile([F_in, F_out], DT.bfloat16)
        wf32 = cpool.tile([F_in, F_out], DT.float32)
        nc.sync.dma_start(wf32[:], w_hbm[:])
        nc.vector.tensor_copy(w_sb[:], wf32[:])
        ident = cpool.tile([P, P], DT.bfloat16)
        nc.sync.dma_start(ident[:], ident_hbm[:])
        iota = cpool.tile([P, P], DT.bfloat16)
        nc.sync.dma_start(iota[:], iota_hbm[:])

        sp = ctx.enter_context(tc.tile_pool(name="stream", bufs=3))
        mp = ctx.enter_context(tc.tile_pool(name="m", bufs=2))
        gp = ctx.enter_context(tc.tile_pool(name="g", bufs=2))
        ep = ctx.enter_context(tc.tile_pool(name="epi", bufs=2))
        pp = ctx.enter_context(tc.tile_pool(name="ps", bufs=2, space="PSUM"))
        pp2 = ctx.enter_context(tc.tile_pool(name="ps2", bufs=2, space="PSUM"))
        pp3 = ctx.enter_context(tc.tile_pool(name="ps3", bufs=2, space="PSUM"))

        for wi in range(n_win):
            S = sp.tile([P, t_w, C], DT.bfloat16, tag="S")
            nc.sync.dma_start(S[:], stream[wi].rearrange("p (t c) -> p t c", c=C))
            lk = ep.tile([P, t_w, 1], DT.float32, tag="lk")
            nc.vector.tensor_scalar_mul(lk[:], S[:, :, F_in + 1:F_in + 2], NEG_SLOPE)
            nc.vector.tensor_tensor(out=lk[:], in0=lk[:],
                                    in1=S[:, :, F_in + 1:F_in + 2],
                                    op=mybir.AluOpType.max)
            wcol = ep.tile([P, t_w, 1], DT.bfloat16, tag="wcol")
            nc.scalar.activation(wcol[:], lk[:], AF.Exp)
            M = mp.tile([P, t_w, P], DT.bfloat16, tag="M")
            nc.vector.tensor_tensor(
                out=M[:],
                in0=iota[:, None, :].broadcast_to([P, t_w, P]),
                in1=S[:, :, F_in + 2:F_in + 3].broadcast_to([P, t_w, P]),
                op=mybir.AluOpType.is_equal)
            Gw = gp.tile([P, t_w, R], DT.bfloat16, tag="Gw")
            nc.vector.tensor_tensor(
                out=Gw[:],
                in0=S[:, :, 0:R],
                in1=wcol[:].broadcast_to([P, t_w, R]),
                op=mybir.AluOpType.mult)
            ps = pp.tile([P, R], DT.float32, tag="ps")
            for t in range(t_w):
                nc.tensor.matmul(ps[:], lhsT=M[:, t, :], rhs=Gw[:, t, :],
                                 start=(t == 0), stop=(t == t_w - 1))
            zinv = ep.tile([P, 1], DT.float32, tag="zinv")
            nc.vector.reciprocal(zinv[:], ps[:, F_in:F_in + 1])
            aggn = ep.tile([P, F_in], DT.bfloat16, tag="aggn")
            nc.vector.tensor_scalar_mul(aggn[:], ps[:, 0:F_in], zinv[:])
            ps2 = pp2.tile([F_in, P], DT.bfloat16, tag="ps2")
            nc.tensor.transpose(out=ps2[:], in_=aggn[:], identity=ident[:])
            aggnT = ep.tile([F_in, P], DT.bfloat16, tag="aggnT")
            nc.vector.tensor_copy(aggnT[:], ps2[:])
            ps3 = pp3.tile([F_out, P], DT.float32, tag="ps3")
            nc.tensor.matmul(ps3[:], lhsT=w_sb[:], rhs=aggnT[:],
                             start=True, stop=True)
            o = ep.tile([F_out, P], DT.float32, tag="o")
            if relu:
                nc.scalar.activation(o[:], ps3[:], AF.Relu)
            else:
                nc.vector.tensor_copy(o[:], ps3[:])
            nc.sync.dma_start(outT[:, wi * P:(wi + 1) * P], o[:])
    nc.compile()
    return nc


def _get_layer(F_in, F_out, relu):
    key = (F_in, F_out, relu)
    if key not in _CACHE:
        _CACHE[key] = _build_layer(F_in, F_out, WPC, TW, relu)
    return _CACHE[key]


def _build_null(F_in, F_out, n_win, t_w):
    """Same I/O signature as a layer, trivial body — for timing calibration."""
    C = F_in + 4
    nc = bacc.Bacc("TRN2", target_bir_lowering=False, debug=False,
                   num_devices=NCORES)
    nc.dram_tensor("stream", [n_win, P, t_w * C], DT.bfloat16,
                   kind="ExternalInput").ap()
    w_hbm = nc.dram_tensor("w", [F_in, F_out], DT.float32,
                           kind="ExternalInput").ap()
    nc.dram_tensor("ident", [P, P], DT.bfloat16, kind="ExternalInput").ap()
    nc.dram_tensor("iota", [P, P], DT.bfloat16, kind="ExternalInput").ap()
    outT = nc.dram_tensor("outT", [F_out, n_win * P], DT.float32,
                          kind="ExternalOutput").ap()
    with tile.TileContext(nc) as tc, ExitStack() as ctx:
        pool = ctx.enter_context(tc.tile_pool(name="sb", bufs=1))
        t = pool.tile([F_in, F_out], DT.float32)
        nc.sync.dma_start(t[:], w_hbm[:])
        o = pool.tile([F_out, P], DT.float32)
        nc.vector.memset(o[:], 0.0)
        nc.sync.dma_start(outT[:, 0:P], o[:])
    nc.compile()
    return nc


def _get_layer_null(F_in):
    key = ("null", F_in)
    if key not in _CACHE:
        _CACHE[key] = _build_null(F_in, NHID, WPC, TW)
    return _CACHE[key]


# ------------------------------------------------------------------ host ----

def _make_consts():
    ident = np.eye(P, dtype=np.float32).astype(BF16)
    iota = np.broadcast_to(np.arange(P, dtype=np.float32), (P, P)).astype(BF16).copy()
    return ident, iota


def _prep_graph(edge_index):
    """Per-core slot assignment. Returns list of dicts with slot_src (int64),
    slot_dst (int64 global), dstloc (f32, -1 pad)."""
    src = np.concatenate([edge_index[0], np.arange(N, dtype=edge_index.dtype)])
    dst = np.concatenate([edge_index[1], np.arange(N, dtype=edge_index.dtype)])
    src = src.astype(np.int64)
    dst = dst.astype(np.int64)
    owner = dst // NSH
    cores = []
    for c in range(NCORES):
        sel = owner == c
        s_c = src[sel]
        d_c = dst[sel] - c * NSH          # local 0..12499
        order = np.argsort(d_c, kind="stable")
        s_c, d_c = s_c[order], d_c[order]
        win = d_c // P
        # slot position within window: running index over the sorted-by-dst list
        start = np.searchsorted(win, np.arange(WPC))
        cnt = np.diff(np.append(start, len(d_c)))
        if cnt.max() > TW * P - P:  # leave room for pad-node fake edges
            raise RuntimeError(f"window overflow: {cnt.max()}")
        pos = np.arange(len(d_c)) - start[win]
        slot = win * (TW * P) + pos
        slot_src = np.zeros(SLOTS, dtype=np.int64)
        slot_dst = np.zeros(SLOTS, dtype=np.int64)
        dstloc = np.full(SLOTS, -1.0, dtype=np.float32)
        slot_src[slot] = s_c
        slot_dst[slot] = d_c + c * NSH
        dstloc[slot] = d_c % P
        # fake self-edge for padded node ids (12500..13311) so z > 0
        padn = np.arange(NSH, NODES_PAD)
        pw = padn // P
        fake_slot = pw * (TW * P) + cnt[pw] + (padn - pw * P)
        # place fakes after real edges of their window (cnt < TW*P - P guaranteed)
        slot_src[fake_slot] = 0
        slot_dst[fake_slot] = 0
        dstloc[fake_slot] = padn % P
        cores.append(dict(slot_src=slot_src, slot_dst=slot_dst, dstloc=dstloc))
    return cores


def _build_stream(feat_table, pre_all, core):
    """feat_table [N, F] f32; pre_all = s[src]+d[dst] per slot [SLOTS] f32."""
    F = feat_table.shape[1]
    C = F + 4
    st = np.zeros((SLOTS, C), dtype=np.float32)
    st[:, 0:F] = feat_table[core["slot_src"]]
    st[:, F] = 1.0
    st[:, F + 1] = pre_all
    st[:, F + 2] = core["dstloc"]
    st = st.reshape(WPC, TW, P, C).transpose(0, 2, 1, 3).reshape(WPC, P, TW * C)
    return st.astype(BF16)


def _run_layer(nc_layer, streams, Wmat, ident, iota, F_out):
    in_maps = [{"stream": streams[c], "w": np.ascontiguousarray(Wmat, dtype=np.float32),
                "ident": ident, "iota": iota} for c in range(NCORES)]
    res = run_bass_kernel_spmd(nc_layer, in_maps, core_ids=list(range(NCORES)))
    outs = []
    for c in range(NCORES):
        outT = res.results[c]["outT"]          # [F_out, 13312]
        outs.append(outT[:, :NSH].T)           # [12500, F_out]
    return np.concatenate(outs, axis=0)        # [100000, F_out]


def kernel(x, W1, att_src1, att_dst1, W2, att_src2, att_dst2, edge_index):
    x = np.asarray(x, dtype=np.float32)
    W1 = np.asarray(W1, dtype=np.float32)
    W2 = np.asarray(W2, dtype=np.float32)
    att_src1 = np.asarray(att_src1, dtype=np.float32)
    att_dst1 = np.asarray(att_dst1, dtype=np.float32)
    att_src2 = np.asarray(att_src2, dtype=np.float32)
    att_dst2 = np.asarray(att_dst2, dtype=np.float32)
    edge_index = np.asarray(edge_index)

    cores = _prep_graph(edge_index)
    ident, iota = _make_consts()

    ncA = _get_layer(NFEAT, NHID, True)
    ncB = _get_layer(NHID, NHID, False)

    # layer 1: aggregate raw x rows (W1 applied post-aggregation by linearity)
    s1 = x @ (W1 @ att_src1)
    d1 = x @ (W1 @ att_dst1)
    streams = []
    for c in cores:
        pre = s1[c["slot_src"]] + d1[c["slot_dst"]]
        streams.append(_build_stream(x, pre, c))
    h = _run_layer(ncA, streams, W1, ident, iota, NHID)

    # layer 2
    s2 = h @ (W2 @ att_src2)
    d2 = h @ (W2 @ att_dst2)
    streams = []
    for c in cores:
        pre = s2[c["slot_src"]] + d2[c["slot_dst"]]
        streams.append(_build_stream(h, pre, c))
    out = _run_layer(ncB, streams, W2, ident, iota, NHID)
    return out.astype(np.float32)



# revision 3
# speedup vs baseline: 6462.9702x; 6462.9702x over previous
"""Trainium2 8-core kernel for 2-layer GAT (nn_DiGCN_65335042507185).

Design (v2):
  Nodes are partitioned across 8 cores by dst (12500/core). Per core, dst
  nodes are bin-packed by in-degree into 392 groups of 32 nodes; each group
  owns 5 edge-tiles of 128 slots (640 capacity). Four NEFFs per call:

    A  (lin, F=128): xs1 = x@W1 and attention preacts s1,d1 on device.
    B  (agg, relu):  layer-1 edge softmax + one-hot aggregation.
    B2 (lin, F=64):  xs2 = h@W2 and preacts s2,d2 on device.
    C  (agg):        layer-2 aggregation -> final embeddings.

  The agg NEFF streams host-gathered xs[src] rows (64 feats + ones col,
  bf16) plus per-slot preact/dstloc. On device: LeakyReLU+exp (softmax
  numerator), a 32-wide one-hot built at DVE 2x mode ([P,G,TC] layout with
  materialized iota), weight folded into the one-hot, 32-col TensorE
  matmuls with tile_position packing 4 groups per PSUM bank, and
  normalization Relu(z^-1 * agg) on ScalarE. Host does graph partitioning,
  slot layout, gathers (halo exchange surrogate), and resharding only.
"""
import sys
for _p in ("/opt/trn_rl_repo", "/root/.axon_site/_ro/trn_rl_repo"):
    if _p not in sys.path:
        sys.path.insert(0, _p)

import numpy as np
import ml_dtypes
from contextlib import ExitStack

import concourse.bass as bass
import concourse.bacc as bacc
import concourse.mybir as mybir
import concourse.tile as tile
from concourse.bass_utils import run_bass_kernel_spmd

P = 128
N = 100_000
NFEAT = 128
NHID = 64
NEG_SLOPE = 0.2
NCORES = 8
NSH = 12500                  # real nodes per core
G = 32                       # dst nodes per group (one-hot width)
TPG = 5                      # tiles per group (640 edge slots capacity)
NGRP = 392                   # groups per core
NODES_PAD = NGRP * G         # 12544 node slots per core
NT = NGRP * TPG              # 1960 tiles per core
NSLOT = NT * P               # 250880 edge slots per core
GPC = 28                     # groups per chunk
TC = GPC * TPG               # 140 tiles per chunk
NCHUNK = NGRP // GPC         # 14
NPS = NGRP // 4              # 98 psum tiles (4 groups each)
PPC = GPC // 4               # 7 psum tiles per chunk
CS = 65                      # stream cols: 64 feats + ones
AF = mybir.ActivationFunctionType
DT = mybir.dt
ALU = mybir.AluOpType
BF16 = ml_dtypes.bfloat16

_CACHE = {}


# ---------------------------------------------------------------- device ----

def _build_lin(F):
    """xs = x@W plus preacts s,d. In: xT [F,NODES_PAD] bf16, W [F,64] bf16,
    WT [64,F] bf16, apair [64,2] bf16. Out: xs_sd [66, NODES_PAD] bf16
    (rows 0:64 = xs^T, 64:66 = s,d)."""
    nc = bacc.Bacc("TRN2", target_bir_lowering=False, debug=False,
                   num_devices=NCORES)
    xT = nc.dram_tensor("xT", [F, NODES_PAD], DT.bfloat16,
                        kind="ExternalInput").ap()
    w_h = nc.dram_tensor("w", [F, NHID], DT.bfloat16, kind="ExternalInput").ap()
    wT_h = nc.dram_tensor("wT", [NHID, F], DT.bfloat16, kind="ExternalInput").ap()
    ap_h = nc.dram_tensor("apair", [NHID, 2], DT.bfloat16, kind="ExternalInput").ap()
    out_h = nc.dram_tensor("xs_sd", [NHID + 2, NODES_PAD], DT.bfloat16,
                           kind="ExternalOutput").ap()
    NTILE = NODES_PAD // P          # 98
    CHT = 14                        # node tiles per input DMA chunk
    with tile.TileContext(nc) as tc, ExitStack() as ctx:
        cpool = ctx.enter_context(tc.tile_pool(name="consts", bufs=1))
        wcat = cpool.tile([F, NHID + 2], DT.bfloat16)
        nc.sync.dma_start(wcat[:, 0:NHID], w_h[:])
        wT = cpool.tile([NHID, F], DT.bfloat16)
        nc.sync.dma_start(wT[:], wT_h[:])
        apair = cpool.tile([NHID, 2], DT.bfloat16)
        nc.sync.dma_start(apair[:], ap_h[:])
        with tc.tile_pool(name="va", bufs=1, space="PSUM") as vpool:
            va_ps = vpool.tile([F, 2], DT.float32)
            nc.tensor.matmul(va_ps[:], lhsT=wT[:], rhs=apair[:],
                             start=True, stop=True)
            nc.vector.tensor_copy(wcat[:, NHID:NHID + 2], va_ps[:])

        stage = cpool.tile([NHID + 2, NODES_PAD], DT.bfloat16)
        xp = ctx.enter_context(tc.tile_pool(name="x", bufs=2))
        pp = ctx.enter_context(tc.tile_pool(name="ps", bufs=8, space="PSUM"))
        for ci in range(NTILE // CHT):
            xt = xp.tile([F, CHT, P], DT.bfloat16, tag="xt")
            nc.sync.dma_start(
                xt[:], xT[:, ci * CHT * P:(ci + 1) * CHT * P]
                .rearrange("f (t p) -> f t p", p=P))
            for k in range(CHT):
                ti = ci * CHT + k
                ps = pp.tile([NHID + 2, P], DT.float32, tag="ps")
                nc.tensor.matmul(ps[:], lhsT=wcat[:], rhs=xt[:, k, :],
                                 start=True, stop=True)
                if k % 2 == 0:
                    nc.vector.tensor_copy(stage[:, ti * P:(ti + 1) * P], ps[:])
                else:
                    nc.scalar.activation(stage[:, ti * P:(ti + 1) * P], ps[:],
                                         AF.Copy)
        nc.sync.dma_start(out_h[:], stage[:])
    nc.compile()
    return nc


def _build_agg(relu, f32_out):
    """One GAT aggregation layer over the packed edge stream."""
    nc = bacc.Bacc("TRN2", target_bir_lowering=False, debug=False,
                   num_devices=NCORES)
    feats = nc.dram_tensor("feats", [P, NT, CS], DT.bfloat16,
                           kind="ExternalInput").ap()
    pre_h = nc.dram_tensor("pre", [P, NT], DT.bfloat16, kind="ExternalInput").ap()
    dst_h = nc.dram_tensor("dstloc", [P, NT], DT.bfloat16,
                           kind="ExternalInput").ap()
    iota_h = nc.dram_tensor("iota", [P, G, TC], DT.bfloat16,
                            kind="ExternalInput").ap()
    odt = DT.float32 if f32_out else DT.bfloat16
    out_h = nc.dram_tensor("out", [P, NPS, NHID], odt, kind="ExternalOutput").ap()

    with tile.TileContext(nc) as tc, ExitStack() as ctx:
        cpool = ctx.enter_context(tc.tile_pool(name="consts", bufs=1))
        iota = cpool.tile([P, G, TC], DT.bfloat16)
        nc.sync.dma_start(iota[:], iota_h[:])

        sp = ctx.enter_context(tc.tile_pool(name="stream", bufs=2))
        mp = ctx.enter_context(tc.tile_pool(name="meta", bufs=2))
        ep = ctx.enter_context(tc.tile_pool(name="edge", bufs=2))
        mwp = ctx.enter_context(tc.tile_pool(name="mw", bufs=2))
        op = ctx.enter_context(tc.tile_pool(name="out", bufs=2))
        zp = ctx.enter_context(tc.tile_pool(name="z", bufs=8))
        pp = ctx.enter_context(tc.tile_pool(name="ps", bufs=8, space="PSUM"))

        for ci in range(NCHUNK):
            t0 = ci * TC
            S = sp.tile([P, TC, CS], DT.bfloat16, tag="S")
            nc.sync.dma_start(S[:], feats[:, t0:t0 + TC, :])
            pre = mp.tile([P, TC], DT.bfloat16, tag="pre")
            nc.sync.dma_start(pre[:], pre_h[:, t0:t0 + TC])
            dstl = mp.tile([P, TC], DT.bfloat16, tag="dstl")
            nc.sync.dma_start(dstl[:], dst_h[:, t0:t0 + TC])

            lk = ep.tile([P, TC], DT.float32, tag="lk")
            nc.vector.tensor_scalar(out=lk[:], in0=pre[:], scalar1=NEG_SLOPE,
                                    scalar2=None, op0=ALU.mult)
            nc.vector.tensor_tensor(out=lk[:], in0=lk[:], in1=pre[:], op=ALU.max)
            w = ep.tile([P, TC], DT.bfloat16, tag="w")
            nc.scalar.activation(w[:], lk[:], AF.Exp)

            M = mwp.tile([P, G, TC], DT.bfloat16, tag="M")
            nc.vector.tensor_tensor(
                out=M[:], in0=dstl[:, None, :].broadcast_to([P, G, TC]),
                in1=iota[:], op=ALU.is_equal)
            Mw = mwp.tile([P, G, TC], DT.bfloat16, tag="Mw")
            nc.vector.tensor_tensor(
                out=Mw[:], in0=M[:], in1=w[:, None, :].broadcast_to([P, G, TC]),
                op=ALU.mult)

            outsb = op.tile([P, PPC, NHID], odt, tag="outsb")
            for k in range(PPC):
                ps = pp.tile([P, CS], DT.float32, tag="ps")
                for j in range(4):
                    gl = k * 4 + j
                    tt = gl * TPG
                    for t in range(TPG):
                        nc.tensor.matmul(ps[G * j:G * (j + 1), :],
                                         lhsT=Mw[:, :, tt + t],
                                         rhs=S[:, tt + t, :],
                                         start=(t == 0), stop=(t == TPG - 1),
                                         tile_position=(0, G * j))
                zinv = zp.tile([P, 1], DT.float32, tag="zinv")
                nc.vector.reciprocal(zinv[:], ps[:, NHID:NHID + 1])
                nc.scalar.activation(outsb[:, k, :], ps[:, 0:NHID],
                                     AF.Relu if relu else AF.Copy,
                                     scale=zinv[:])
            nc.sync.dma_start(out_h[:, ci * PPC:(ci + 1) * PPC, :], outsb[:])
    nc.compile()
    return nc


def _get(key, builder, *a):
    if key not in _CACHE:
        _CACHE[key] = builder(*a)
    return _CACHE[key]


# ------------------------------------------------------------------ host ----

def _bin_pack(deg):
    """LPT: assign NSH nodes to NGRP bins of exactly G slots, load<=TPG*P.
    Returns perm [NGRP*G] int32 (node id or -1 for pad)."""
    import heapq
    order = np.argsort(-deg, kind="stable")
    heap = [(0, g) for g in range(NGRP)]
    heapq.heapify(heap)
    bins = [[] for _ in range(NGRP)]
    spill = []
    for n in order:
        d = int(deg[n])
        load, g = heapq.heappop(heap)
        while len(bins[g]) >= G:
            load, g = heapq.heappop(heap)
        bins[g].append(n)
        nl = load + d
        if nl > TPG * P:
            raise RuntimeError(f"bin overflow {nl}")
        if len(bins[g]) < G:
            heapq.heappush(heap, (nl, g))
        else:
            spill.append((nl, g))
    perm = np.full(NGRP * G, -1, dtype=np.int64)
    for g, lst in enumerate(bins):
        perm[g * G:g * G + len(lst)] = lst
    return perm


def _prep_graph(edge_index):
    """Per-core slot layout. Returns list of dicts."""
    ei = np.asarray(edge_index)
    src = np.concatenate([ei[0], np.arange(N, dtype=ei.dtype)]).astype(np.int64)
    dst = np.concatenate([ei[1], np.arange(N, dtype=ei.dtype)]).astype(np.int64)
    owner = dst // NSH
    cores = []
    for c in range(NCORES):
        sel = owner == c
        s_c = src[sel]
        d_c = dst[sel] - c * NSH                     # local dst 0..12499
        deg = np.bincount(d_c, minlength=NSH)
        perm = _bin_pack(deg)                        # [12544] node or -1
        # node -> (group, j)
        slot_of_node = np.full(NSH, -1, dtype=np.int64)
        valid = perm >= 0
        slot_of_node[perm[valid]] = np.nonzero(valid)[0]
        key = slot_of_node[d_c]                      # g*32+j per edge
        order = np.argsort(key, kind="stable")
        s_c, d_c, key = s_c[order], d_c[order], key[order]
        grp = key // G
        # position within group: running index
        gstart = np.searchsorted(grp, np.arange(NGRP))
        cnt = np.diff(np.append(gstart, len(grp)))
        if cnt.max() > TPG * P:
            raise RuntimeError(f"group overflow {cnt.max()}")
        pos = np.arange(len(grp)) - gstart[grp]
        slot = grp * (TPG * P) + pos                 # linear slot in [0, NSLOT)
        slot_src = np.zeros(NSLOT, dtype=np.int64)
        slot_dst_g = np.zeros(NSLOT, dtype=np.int64) # global dst per slot
        dstloc = np.zeros(NSLOT, dtype=np.float32)
        wkill = np.full(NSLOT, True)                 # pad slots
        slot_src[slot] = s_c
        slot_dst_g[slot] = d_c + c * NSH
        dstloc[slot] = key % G
        wkill[slot] = False
        cores.append(dict(slot_src=slot_src, slot_dst=slot_dst_g,
                          dstloc=dstloc.astype(BF16), wkill=wkill, perm=perm))
    return cores


def _make_iota():
    i = np.arange(G, dtype=np.float32)[None, :, None]
    return np.broadcast_to(i, (P, G, TC)).astype(BF16).copy()


def _feats_stream(table66, core):
    """table66 [N,65] bf16 (col 64 = 1.0). -> [P, NT, CS] bf16."""
    flat = table66[core["slot_src"]]                 # [NSLOT, 65]
    flat[core["wkill"], 64] = 0                      # ones col 0 on pad slots
    return np.ascontiguousarray(
        flat.reshape(NT, P, CS).transpose(1, 0, 2))


def _meta_streams(pre_f32, core):
    pre = pre_f32.copy()
    pre[core["wkill"]] = -30000.0
    pre = pre.astype(BF16).reshape(NT, P).T.copy()
    dstl = core["dstloc"].reshape(NT, P).T.copy()
    return pre, dstl


def _run_lin(nc_lin, xT_list, W, a_src, a_dst):
    Wb = np.ascontiguousarray(W, dtype=np.float32).astype(BF16)
    WTb = np.ascontiguousarray(W.T, dtype=np.float32).astype(BF16)
    ap = np.stack([a_src, a_dst], axis=1).astype(np.float32).astype(BF16)
    in_maps = [{"xT": xT_list[c], "w": Wb, "wT": WTb, "apair": ap}
               for c in range(NCORES)]
    res = run_bass_kernel_spmd(nc_lin, in_maps, core_ids=list(range(NCORES)))
    # assemble global tables: xs [N,64] bf16 (from cols 0:NSH), s,d [N] f32
    xs = np.empty((N, NHID + 2), dtype=np.float32)
    for c in range(NCORES):
        xs[c * NSH:(c + 1) * NSH] = \
            res.results[c]["xs_sd"][:, :NSH].T.astype(np.float32)
    return xs[:, 0:NHID], xs[:, NHID], xs[:, NHID + 1]


def _run_agg(nc_agg, cores, xs, s, d, iota):
    table66 = np.empty((N, CS), dtype=np.float32)
    table66[:, 0:NHID] = xs
    table66[:, NHID] = 1.0
    table66 = table66.astype(BF16)
    in_maps = []
    for core in cores:
        pre = s[core["slot_src"]] + d[core["slot_dst"]]
        pre_st, dst_st = _meta_streams(pre, core)
        in_maps.append({"feats": _feats_stream(table66, core),
                        "pre": pre_st, "dstloc": dst_st, "iota": iota})
    res = run_bass_kernel_spmd(nc_agg, in_maps, core_ids=list(range(NCORES)))
    # out [P, NPS, 64] -> rows r = pstile*128+p = g*32+j -> node perm[g*32+j]
    full = np.empty((N, NHID), dtype=np.float32)
    for c, core in enumerate(cores):
        o = res.results[c]["out"]                   # [P, NPS, 64]
        rows = o.transpose(1, 0, 2).reshape(NODES_PAD, NHID).astype(np.float32)
        valid = core["perm"] >= 0
        full[c * NSH + core["perm"][valid]] = rows[valid]
    return full


def kernel(x, W1, att_src1, att_dst1, W2, att_src2, att_dst2, edge_index):
    x = np.asarray(x, dtype=np.float32)
    W1 = np.asarray(W1, dtype=np.float32)
    W2 = np.asarray(W2, dtype=np.float32)
    a_s1 = np.asarray(att_src1, dtype=np.float32)
    a_d1 = np.asarray(att_dst1, dtype=np.float32)
    a_s2 = np.asarray(att_src2, dtype=np.float32)
    a_d2 = np.asarray(att_dst2, dtype=np.float32)

    cores = _prep_graph(edge_index)
    iota = _make_iota()

    ncA = _get(("lin", NFEAT), _build_lin, NFEAT)
    ncB2 = _get(("lin", NHID), _build_lin, NHID)
    ncB = _get(("agg", True), _build_agg, True, False)
    ncC = _get(("agg", False), _build_agg, False, True)

    # layer 1
    xb = x.astype(BF16)
    xT_list = []
    for c in range(NCORES):
        xt = np.zeros((NFEAT, NODES_PAD), dtype=BF16)
        xt[:, :NSH] = xb[c * NSH:(c + 1) * NSH].T
        xT_list.append(xt)
    xs1, s1, d1 = _run_lin(ncA, xT_list, W1, a_s1, a_d1)
    h = _run_agg(ncB, cores, xs1, s1, d1, iota)

    # layer 2
    hb = h.astype(BF16)
    hT_list = []
    for c in range(NCORES):
        ht = np.zeros((NHID, NODES_PAD), dtype=BF16)
        ht[:, :NSH] = hb[c * NSH:(c + 1) * NSH].T
        hT_list.append(ht)
    xs2, s2, d2 = _run_lin(ncB2, hT_list, W2, a_s2, a_d2)
    out = _run_agg(ncC, cores, xs2, s2, d2, iota)
    return out.astype(np.float32)


# revision 7
# speedup vs baseline: 6774.9788x; 1.0483x over previous
"""Trainium2 8-core kernel for 2-layer GAT (nn_DiGCN_65335042507185).

Design (v2):
  Nodes are partitioned across 8 cores by dst (12500/core). Per core, dst
  nodes are bin-packed by in-degree into 392 groups of 32 nodes; each group
  owns 5 edge-tiles of 128 slots (640 capacity). Four NEFFs per call:

    A  (lin, F=128): xs1 = x@W1 and attention preacts s1,d1 on device.
    B  (agg, relu):  layer-1 edge softmax + one-hot aggregation.
    B2 (lin, F=64):  xs2 = h@W2 and preacts s2,d2 on device.
    C  (agg):        layer-2 aggregation -> final embeddings.

  The agg NEFF streams host-gathered xs[src] rows (64 feats + ones col,
  bf16) plus per-slot preact/dstloc. On device: LeakyReLU+exp (softmax
  numerator), a 32-wide one-hot built at DVE 2x mode ([P,G,TC] layout with
  materialized iota), weight folded into the one-hot, 32-col TensorE
  matmuls with tile_position packing 4 groups per PSUM bank, and
  normalization Relu(z^-1 * agg) on ScalarE. Host does graph partitioning,
  slot layout, gathers (halo exchange surrogate), and resharding only.
"""
import sys
for _p in ("/opt/trn_rl_repo", "/root/.axon_site/_ro/trn_rl_repo"):
    if _p not in sys.path:
        sys.path.insert(0, _p)

import numpy as np
import ml_dtypes
from contextlib import ExitStack

import concourse.bass as bass
import concourse.bacc as bacc
import concourse.mybir as mybir
import concourse.tile as tile
from concourse.bass_utils import run_bass_kernel_spmd

P = 128
N = 100_000
NFEAT = 128
NHID = 64
NEG_SLOPE = 0.2
NCORES = 8
NSH = 12500                  # real nodes per core
G = 32                       # dst nodes per group (one-hot width)
TPG = 5                      # tiles per group (640 edge slots capacity)
NGRP = 392                   # groups per core
NODES_PAD = NGRP * G         # 12544 node slots per core
NT = NGRP * TPG              # 1960 tiles per core
NSLOT = NT * P               # 250880 edge slots per core
GPC = 28                     # groups per chunk
TC = GPC * TPG               # 140 tiles per chunk
NCHUNK = NGRP // GPC         # 14
NPS = NGRP // 4              # 98 psum tiles (4 groups each)
PPC = GPC // 4               # 7 psum tiles per chunk
CS = 65                      # stream cols: 64 feats + ones
AF = mybir.ActivationFunctionType
DT = mybir.dt
ALU = mybir.AluOpType
BF16 = ml_dtypes.bfloat16
FP8 = False
F8 = ml_dtypes.float8_e4m3

_CACHE = {}


# ---------------------------------------------------------------- device ----

def _build_lin(F):
    """xs = x@W plus preacts s,d. In: xT [F,NODES_PAD] bf16, W [F,64] bf16,
    WT [64,F] bf16, apair [64,2] bf16. Out: xs_sd [66, NODES_PAD] bf16
    (rows 0:64 = xs^T, 64:66 = s,d)."""
    nc = bacc.Bacc("TRN2", target_bir_lowering=False, debug=False,
                   num_devices=NCORES)
    xT = nc.dram_tensor("xT", [F, NODES_PAD], DT.bfloat16,
                        kind="ExternalInput").ap()
    w_h = nc.dram_tensor("w", [F, NHID], DT.bfloat16, kind="ExternalInput").ap()
    wT_h = nc.dram_tensor("wT", [NHID, F], DT.bfloat16, kind="ExternalInput").ap()
    ap_h = nc.dram_tensor("apair", [NHID, 2], DT.bfloat16, kind="ExternalInput").ap()
    out_h = nc.dram_tensor("xs_sd", [NHID + 2, NODES_PAD], DT.bfloat16,
                           kind="ExternalOutput").ap()
    NTILE = NODES_PAD // P          # 98
    CHT = 14                        # node tiles per input DMA chunk
    with tile.TileContext(nc) as tc, ExitStack() as ctx:
        cpool = ctx.enter_context(tc.tile_pool(name="consts", bufs=1))
        wcat = cpool.tile([F, NHID + 2], DT.bfloat16)
        nc.sync.dma_start(wcat[:, 0:NHID], w_h[:])
        wT = cpool.tile([NHID, F], DT.bfloat16)
        nc.sync.dma_start(wT[:], wT_h[:])
        apair = cpool.tile([NHID, 2], DT.bfloat16)
        nc.sync.dma_start(apair[:], ap_h[:])
        with tc.tile_pool(name="va", bufs=1, space="PSUM") as vpool:
            va_ps = vpool.tile([F, 2], DT.float32)
            nc.tensor.matmul(va_ps[:], lhsT=wT[:], rhs=apair[:],
                             start=True, stop=True)
            nc.vector.tensor_copy(wcat[:, NHID:NHID + 2], va_ps[:])

        stage = cpool.tile([NHID + 2, NODES_PAD], DT.bfloat16)
        xp = ctx.enter_context(tc.tile_pool(name="x", bufs=2))
        pp = ctx.enter_context(tc.tile_pool(name="ps", bufs=8, space="PSUM"))
        MMW = 2 * P                       # rhs cols per matmul
        for ci in range(NTILE // CHT):
            xt = xp.tile([F, CHT * P], DT.bfloat16, tag="xt")
            nc.sync.dma_start(xt[:], xT[:, ci * CHT * P:(ci + 1) * CHT * P])
            for k in range(CHT * P // MMW):
                c0 = ci * CHT * P + k * MMW
                ps = pp.tile([NHID + 2, MMW], DT.float32, tag="ps")
                nc.tensor.matmul(ps[:], lhsT=wcat[:],
                                 rhs=xt[:, k * MMW:(k + 1) * MMW],
                                 start=True, stop=True)
                if k % 2 == 0:
                    nc.vector.tensor_copy(stage[:, c0:c0 + MMW], ps[:])
                else:
                    nc.scalar.activation(stage[:, c0:c0 + MMW], ps[:], AF.Copy)
        nc.sync.dma_start(out_h[:], stage[:])
    nc.compile()
    return nc


def _build_agg(relu, f32_out):
    """One GAT aggregation layer over the packed edge stream."""
    nc = bacc.Bacc("TRN2", target_bir_lowering=False, debug=False,
                   num_devices=NCORES)
    sdt = DT.float8e4 if FP8 else DT.bfloat16
    feats = nc.dram_tensor("feats", [P, NT, CS], sdt,
                           kind="ExternalInput").ap()
    pre_h = nc.dram_tensor("pre", [P, NT], DT.bfloat16, kind="ExternalInput").ap()
    dst_h = nc.dram_tensor("dstloc", [P, NT], DT.bfloat16,
                           kind="ExternalInput").ap()
    iota_h = nc.dram_tensor("iota", [P, G, TC], DT.bfloat16,
                            kind="ExternalInput").ap()
    odt = DT.float32 if f32_out else DT.bfloat16
    out_h = nc.dram_tensor("out", [P, NPS, NHID], odt, kind="ExternalOutput").ap()

    with tile.TileContext(nc) as tc, ExitStack() as ctx:
        cpool = ctx.enter_context(tc.tile_pool(name="consts", bufs=1))
        iota = cpool.tile([P, G, TC], DT.bfloat16)
        nc.sync.dma_start(iota[:], iota_h[:])

        sp = ctx.enter_context(tc.tile_pool(name="stream", bufs=3))
        mp = ctx.enter_context(tc.tile_pool(name="meta", bufs=2))
        ep = ctx.enter_context(tc.tile_pool(name="edge", bufs=2))
        mwp = ctx.enter_context(tc.tile_pool(name="mw", bufs=3))
        op = ctx.enter_context(tc.tile_pool(name="out", bufs=2))
        zp = ctx.enter_context(tc.tile_pool(name="z", bufs=8))
        pp = ctx.enter_context(tc.tile_pool(name="ps", bufs=8, space="PSUM"))

        def _evac(p):
            pl, pci = p
            outsb = op.tile([P, PPC, NHID], odt, tag="outsb")
            for k, ps in enumerate(pl):
                zinv = zp.tile([P, 1], DT.float32, tag="zinv")
                nc.vector.reciprocal(zinv[:], ps[:, NHID:NHID + 1])
                nc.scalar.activation(outsb[:, k, :], ps[:, 0:NHID],
                                     AF.Relu if relu else AF.Copy,
                                     scale=zinv[:])
            nc.sync.dma_start(out_h[:, pci * PPC:(pci + 1) * PPC, :], outsb[:])

        pend = None
        for ci in range(NCHUNK):
            t0 = ci * TC
            S = sp.tile([P, TC, CS], sdt, tag="S")
            nc.sync.dma_start(S[:], feats[:, t0:t0 + TC, :])
            pre = mp.tile([P, TC], DT.bfloat16, tag="pre")
            nc.sync.dma_start(pre[:], pre_h[:, t0:t0 + TC])
            dstl = mp.tile([P, TC], DT.bfloat16, tag="dstl")
            nc.sync.dma_start(dstl[:], dst_h[:, t0:t0 + TC])

            lk = ep.tile([P, TC], DT.float32, tag="lk")
            nc.vector.tensor_scalar(out=lk[:], in0=pre[:], scalar1=NEG_SLOPE,
                                    scalar2=None, op0=ALU.mult)
            nc.vector.tensor_tensor(out=lk[:], in0=lk[:], in1=pre[:], op=ALU.max)
            w = ep.tile([P, TC], DT.bfloat16, tag="w")
            nc.scalar.activation(w[:], lk[:], AF.Exp)

            M = mwp.tile([P, G, TC], DT.bfloat16, tag="M")
            nc.vector.tensor_tensor(
                out=M[:], in0=dstl[:, None, :].broadcast_to([P, G, TC]),
                in1=iota[:], op=ALU.is_equal)
            Mw = mwp.tile([P, G, TC], DT.bfloat16, tag="Mw")
            nc.vector.tensor_tensor(
                out=Mw[:], in0=M[:], in1=w[:, None, :].broadcast_to([P, G, TC]),
                op=ALU.mult)

            if pend is not None:
                _evac(pend)
            ps_list = []
            for k in range(PPC):
                ps = pp.tile([P, CS], DT.float32, tag="ps")
                for j in range(4):
                    gl = k * 4 + j
                    tt = gl * TPG
                    for t in range(TPG):
                        nc.tensor.matmul(ps[G * j:G * (j + 1), :],
                                         lhsT=Mw[:, :, tt + t],
                                         rhs=S[:, tt + t, :],
                                         start=(t == 0), stop=(t == TPG - 1),
                                         tile_position=(0, G * j))
                ps_list.append(ps)
            pend = (ps_list, ci)
        _evac(pend)
    nc.compile()
    return nc


def _get(key, builder, *a):
    if key not in _CACHE:
        _CACHE[key] = builder(*a)
    return _CACHE[key]


# ------------------------------------------------------------------ host ----

def _bin_pack(deg):
    """LPT: assign NSH nodes to NGRP bins of exactly G slots, load<=TPG*P.
    Returns perm [NGRP*G] int32 (node id or -1 for pad)."""
    import heapq
    order = np.argsort(-deg, kind="stable")
    heap = [(0, g) for g in range(NGRP)]
    heapq.heapify(heap)
    bins = [[] for _ in range(NGRP)]
    spill = []
    for n in order:
        d = int(deg[n])
        load, g = heapq.heappop(heap)
        while len(bins[g]) >= G:
            load, g = heapq.heappop(heap)
        bins[g].append(n)
        nl = load + d
        if nl > TPG * P:
            raise RuntimeError(f"bin overflow {nl}")
        if len(bins[g]) < G:
            heapq.heappush(heap, (nl, g))
        else:
            spill.append((nl, g))
    perm = np.full(NGRP * G, -1, dtype=np.int64)
    for g, lst in enumerate(bins):
        perm[g * G:g * G + len(lst)] = lst
    return perm


def _prep_graph(edge_index):
    """Per-core slot layout. Returns list of dicts."""
    ei = np.asarray(edge_index)
    src = np.concatenate([ei[0], np.arange(N, dtype=ei.dtype)]).astype(np.int64)
    dst = np.concatenate([ei[1], np.arange(N, dtype=ei.dtype)]).astype(np.int64)
    owner = dst // NSH
    cores = []
    for c in range(NCORES):
        sel = owner == c
        s_c = src[sel]
        d_c = dst[sel] - c * NSH                     # local dst 0..12499
        deg = np.bincount(d_c, minlength=NSH)
        perm = _bin_pack(deg)                        # [12544] node or -1
        # node -> (group, j)
        slot_of_node = np.full(NSH, -1, dtype=np.int64)
        valid = perm >= 0
        slot_of_node[perm[valid]] = np.nonzero(valid)[0]
        key = slot_of_node[d_c]                      # g*32+j per edge
        order = np.argsort(key, kind="stable")
        s_c, d_c, key = s_c[order], d_c[order], key[order]
        grp = key // G
        # position within group: running index
        gstart = np.searchsorted(grp, np.arange(NGRP))
        cnt = np.diff(np.append(gstart, len(grp)))
        if cnt.max() > TPG * P:
            raise RuntimeError(f"group overflow {cnt.max()}")
        pos = np.arange(len(grp)) - gstart[grp]
        slot = grp * (TPG * P) + pos                 # linear slot in [0, NSLOT)
        slot_src = np.zeros(NSLOT, dtype=np.int64)
        slot_dst_g = np.zeros(NSLOT, dtype=np.int64) # global dst per slot
        dstloc = np.zeros(NSLOT, dtype=np.float32)
        wkill = np.full(NSLOT, True)                 # pad slots
        slot_src[slot] = s_c
        slot_dst_g[slot] = d_c + c * NSH
        dstloc[slot] = key % G
        wkill[slot] = False
        cores.append(dict(slot_src=slot_src, slot_dst=slot_dst_g,
                          dstloc=dstloc.astype(BF16), wkill=wkill, perm=perm))
    return cores


def _make_iota():
    i = np.arange(G, dtype=np.float32)[None, :, None]
    return np.broadcast_to(i, (P, G, TC)).astype(BF16).copy()


def _feats_stream(table66, core):
    """table66 [N,65] (col 64 = 1.0). -> [P, NT, CS] stream dtype."""
    flat = table66[core["slot_src"]]                 # [NSLOT, 65]
    flat[core["wkill"], 64] = 0                      # ones col 0 on pad slots
    return np.ascontiguousarray(
        flat.reshape(NT, P, CS).transpose(1, 0, 2))


def _meta_streams(pre_f32, core):
    pre = pre_f32.copy()
    pre[core["wkill"]] = -30000.0
    pre = pre.astype(BF16).reshape(NT, P).T.copy()
    dstl = core["dstloc"].reshape(NT, P).T.copy()
    return pre, dstl


def _run_lin(nc_lin, xT_list, W, a_src, a_dst):
    Wb = np.ascontiguousarray(W, dtype=np.float32).astype(BF16)
    WTb = np.ascontiguousarray(W.T, dtype=np.float32).astype(BF16)
    ap = np.stack([a_src, a_dst], axis=1).astype(np.float32).astype(BF16)
    in_maps = [{"xT": xT_list[c], "w": Wb, "wT": WTb, "apair": ap}
               for c in range(NCORES)]
    res = run_bass_kernel_spmd(nc_lin, in_maps, core_ids=list(range(NCORES)))
    # assemble global tables: xs [N,64] bf16 (from cols 0:NSH), s,d [N] f32
    xs = np.empty((N, NHID + 2), dtype=np.float32)
    for c in range(NCORES):
        xs[c * NSH:(c + 1) * NSH] = \
            res.results[c]["xs_sd"][:, :NSH].T.astype(np.float32)
    return xs[:, 0:NHID], xs[:, NHID], xs[:, NHID + 1]


def _run_agg(nc_agg, cores, xs, s, d, iota):
    table66 = np.empty((N, CS), dtype=np.float32)
    table66[:, 0:NHID] = xs
    table66[:, NHID] = 1.0
    table66 = table66.astype(F8 if FP8 else BF16)
    in_maps = []
    for core in cores:
        pre = s[core["slot_src"]] + d[core["slot_dst"]]
        pre_st, dst_st = _meta_streams(pre, core)
        in_maps.append({"feats": _feats_stream(table66, core),
                        "pre": pre_st, "dstloc": dst_st, "iota": iota})
    res = run_bass_kernel_spmd(nc_agg, in_maps, core_ids=list(range(NCORES)))
    # out [P, NPS, 64] -> rows r = pstile*128+p = g*32+j -> node perm[g*32+j]
    full = np.empty((N, NHID), dtype=np.float32)
    for c, core in enumerate(cores):
        o = res.results[c]["out"]                   # [P, NPS, 64]
        rows = o.transpose(1, 0, 2).reshape(NODES_PAD, NHID).astype(np.float32)
        valid = core["perm"] >= 0
        full[c * NSH + core["perm"][valid]] = rows[valid]
    return full


def kernel(x, W1, att_src1, att_dst1, W2, att_src2, att_dst2, edge_index):
    x = np.asarray(x, dtype=np.float32)
    W1 = np.asarray(W1, dtype=np.float32)
    W2 = np.asarray(W2, dtype=np.float32)
    a_s1 = np.asarray(att_src1, dtype=np.float32)
    a_d1 = np.asarray(att_dst1, dtype=np.float32)
    a_s2 = np.asarray(att_src2, dtype=np.float32)
    a_d2 = np.asarray(att_dst2, dtype=np.float32)

    cores = _prep_graph(edge_index)
    iota = _make_iota()

    ncA = _get(("lin", NFEAT), _build_lin, NFEAT)
    ncB2 = _get(("lin", NHID), _build_lin, NHID)
    ncB = _get(("agg", True), _build_agg, True, False)
    ncC = _get(("agg", False), _build_agg, False, True)

    # layer 1
    xb = x.astype(BF16)
    xT_list = []
    for c in range(NCORES):
        xt = np.zeros((NFEAT, NODES_PAD), dtype=BF16)
        xt[:, :NSH] = xb[c * NSH:(c + 1) * NSH].T
        xT_list.append(xt)
    xs1, s1, d1 = _run_lin(ncA, xT_list, W1, a_s1, a_d1)
    h = _run_agg(ncB, cores, xs1, s1, d1, iota)

    # layer 2
    hb = h.astype(BF16)
    hT_list = []
    for c in range(NCORES):
        ht = np.zeros((NHID, NODES_PAD), dtype=BF16)
        ht[:, :NSH] = hb[c * NSH:(c + 1) * NSH].T
        hT_list.append(ht)
    xs2, s2, d2 = _run_lin(ncB2, hT_list, W2, a_s2, a_d2)
    out = _run_agg(ncC, cores, xs2, s2, d2, iota)
    return out.astype(np.float32)


# revision 8
# speedup vs baseline: 7778.9843x; 1.1482x over previous
"""Trainium2 8-core kernel for 2-layer GAT (nn_DiGCN_65335042507185).

Design (v2):
  Nodes are partitioned across 8 cores by dst (12500/core). Per core, dst
  nodes are bin-packed by in-degree into 392 groups of 32 nodes; each group
  owns 5 edge-tiles of 128 slots (640 capacity). Four NEFFs per call:

    A  (lin, F=128): xs1 = x@W1 and attention preacts s1,d1 on device.
    B  (agg, relu):  layer-1 edge softmax + one-hot aggregation.
    B2 (lin, F=64):  xs2 = h@W2 and preacts s2,d2 on device.
    C  (agg):        layer-2 aggregation -> final embeddings.

  The agg NEFF streams host-gathered xs[src] rows (64 feats + ones col,
  bf16) plus per-slot preact/dstloc. On device: LeakyReLU+exp (softmax
  numerator), a 32-wide one-hot built at DVE 2x mode ([P,G,TC] layout with
  materialized iota), weight folded into the one-hot, 32-col TensorE
  matmuls with tile_position packing 4 groups per PSUM bank, and
  normalization Relu(z^-1 * agg) on ScalarE. Host does graph partitioning,
  slot layout, gathers (halo exchange surrogate), and resharding only.
"""
import sys
for _p in ("/opt/trn_rl_repo", "/root/.axon_site/_ro/trn_rl_repo"):
    if _p not in sys.path:
        sys.path.insert(0, _p)

import numpy as np
import ml_dtypes
from contextlib import ExitStack

import concourse.bass as bass
import concourse.bacc as bacc
import concourse.mybir as mybir
import concourse.tile as tile
from concourse.bass_utils import run_bass_kernel_spmd

P = 128
N = 100_000
NFEAT = 128
NHID = 64
NEG_SLOPE = 0.2
NCORES = 8
NSH = 12500                  # real nodes per core
G = 32                       # dst nodes per group (one-hot width)
TPG = 5                      # tiles per group (640 edge slots capacity)
NGRP = 392                   # groups per core
NODES_PAD = NGRP * G         # 12544 node slots per core
NT = NGRP * TPG              # 1960 tiles per core
NSLOT = NT * P               # 250880 edge slots per core
GPC = 28                     # groups per chunk
TC = GPC * TPG               # 140 tiles per chunk
NCHUNK = NGRP // GPC         # 14
NPS = NGRP // 4              # 98 psum tiles (4 groups each)
PPC = GPC // 4               # 7 psum tiles per chunk
CS = 65                      # stream cols: 64 feats + ones
AF = mybir.ActivationFunctionType
DT = mybir.dt
ALU = mybir.AluOpType
BF16 = ml_dtypes.bfloat16
FP8_L1 = True
F8 = ml_dtypes.float8_e4m3

_CACHE = {}


# ---------------------------------------------------------------- device ----

def _build_lin(F):
    """xs = x@W plus preacts s,d. In: xT [F,NODES_PAD] bf16, W [F,64] bf16,
    WT [64,F] bf16, apair [64,2] bf16. Out: xs_sd [66, NODES_PAD] bf16
    (rows 0:64 = xs^T, 64:66 = s,d)."""
    nc = bacc.Bacc("TRN2", target_bir_lowering=False, debug=False,
                   num_devices=NCORES)
    xT = nc.dram_tensor("xT", [F, NODES_PAD], DT.bfloat16,
                        kind="ExternalInput").ap()
    w_h = nc.dram_tensor("w", [F, NHID], DT.bfloat16, kind="ExternalInput").ap()
    wT_h = nc.dram_tensor("wT", [NHID, F], DT.bfloat16, kind="ExternalInput").ap()
    ap_h = nc.dram_tensor("apair", [NHID, 2], DT.bfloat16, kind="ExternalInput").ap()
    out_h = nc.dram_tensor("xs_sd", [NHID + 2, NODES_PAD], DT.bfloat16,
                           kind="ExternalOutput").ap()
    NTILE = NODES_PAD // P          # 98
    CHT = 14                        # node tiles per input DMA chunk
    with tile.TileContext(nc) as tc, ExitStack() as ctx:
        cpool = ctx.enter_context(tc.tile_pool(name="consts", bufs=1))
        wcat = cpool.tile([F, NHID + 2], DT.bfloat16)
        nc.sync.dma_start(wcat[:, 0:NHID], w_h[:])
        wT = cpool.tile([NHID, F], DT.bfloat16)
        nc.sync.dma_start(wT[:], wT_h[:])
        apair = cpool.tile([NHID, 2], DT.bfloat16)
        nc.sync.dma_start(apair[:], ap_h[:])
        with tc.tile_pool(name="va", bufs=1, space="PSUM") as vpool:
            va_ps = vpool.tile([F, 2], DT.float32)
            nc.tensor.matmul(va_ps[:], lhsT=wT[:], rhs=apair[:],
                             start=True, stop=True)
            nc.vector.tensor_copy(wcat[:, NHID:NHID + 2], va_ps[:])

        stage = cpool.tile([NHID + 2, NODES_PAD], DT.bfloat16)
        xp = ctx.enter_context(tc.tile_pool(name="x", bufs=2))
        pp = ctx.enter_context(tc.tile_pool(name="ps", bufs=8, space="PSUM"))
        MMW = 2 * P                       # rhs cols per matmul
        for ci in range(NTILE // CHT):
            xt = xp.tile([F, CHT * P], DT.bfloat16, tag="xt")
            nc.sync.dma_start(xt[:], xT[:, ci * CHT * P:(ci + 1) * CHT * P])
            for k in range(CHT * P // MMW):
                c0 = ci * CHT * P + k * MMW
                ps = pp.tile([NHID + 2, MMW], DT.float32, tag="ps")
                nc.tensor.matmul(ps[:], lhsT=wcat[:],
                                 rhs=xt[:, k * MMW:(k + 1) * MMW],
                                 start=True, stop=True)
                if k % 2 == 0:
                    nc.vector.tensor_copy(stage[:, c0:c0 + MMW], ps[:])
                else:
                    nc.scalar.activation(stage[:, c0:c0 + MMW], ps[:], AF.Copy)
        nc.sync.dma_start(out_h[:], stage[:])
    nc.compile()
    return nc


def _build_agg(relu, f32_out, fp8):
    """One GAT aggregation layer over the packed edge stream."""
    nc = bacc.Bacc("TRN2", target_bir_lowering=False, debug=False,
                   num_devices=NCORES)
    sdt = DT.float8e4 if fp8 else DT.bfloat16
    feats = nc.dram_tensor("feats", [P, NT, CS], sdt,
                           kind="ExternalInput").ap()
    pre_h = nc.dram_tensor("pre", [P, NT], DT.bfloat16, kind="ExternalInput").ap()
    dst_h = nc.dram_tensor("dstloc", [P, NT], DT.bfloat16,
                           kind="ExternalInput").ap()
    iota_h = nc.dram_tensor("iota", [P, G, TC], DT.bfloat16,
                            kind="ExternalInput").ap()
    odt = DT.float32 if f32_out else DT.bfloat16
    out_h = nc.dram_tensor("out", [P, NPS, NHID], odt, kind="ExternalOutput").ap()

    with tile.TileContext(nc) as tc, ExitStack() as ctx:
        cpool = ctx.enter_context(tc.tile_pool(name="consts", bufs=1))
        iota = cpool.tile([P, G, TC], DT.bfloat16)
        nc.sync.dma_start(iota[:], iota_h[:])

        sp = ctx.enter_context(tc.tile_pool(name="stream", bufs=3))
        mp = ctx.enter_context(tc.tile_pool(name="meta", bufs=2))
        ep = ctx.enter_context(tc.tile_pool(name="edge", bufs=2))
        mwp = ctx.enter_context(tc.tile_pool(name="mw", bufs=3))
        op = ctx.enter_context(tc.tile_pool(name="out", bufs=2))
        zp = ctx.enter_context(tc.tile_pool(name="z", bufs=8))
        pp = ctx.enter_context(tc.tile_pool(name="ps", bufs=8, space="PSUM"))

        def _evac(p):
            pl, pci = p
            outsb = op.tile([P, PPC, NHID], odt, tag="outsb")
            for k, ps in enumerate(pl):
                zinv = zp.tile([P, 1], DT.float32, tag="zinv")
                nc.vector.reciprocal(zinv[:], ps[:, NHID:NHID + 1])
                nc.scalar.activation(outsb[:, k, :], ps[:, 0:NHID],
                                     AF.Relu if relu else AF.Copy,
                                     scale=zinv[:])
            nc.sync.dma_start(out_h[:, pci * PPC:(pci + 1) * PPC, :], outsb[:])

        pend = None
        for ci in range(NCHUNK):
            t0 = ci * TC
            S = sp.tile([P, TC, CS], sdt, tag="S")
            nc.sync.dma_start(S[:], feats[:, t0:t0 + TC, :])
            pre = mp.tile([P, TC], DT.bfloat16, tag="pre")
            nc.sync.dma_start(pre[:], pre_h[:, t0:t0 + TC])
            dstl = mp.tile([P, TC], DT.bfloat16, tag="dstl")
            nc.sync.dma_start(dstl[:], dst_h[:, t0:t0 + TC])

            lk = ep.tile([P, TC], DT.float32, tag="lk")
            nc.vector.tensor_scalar(out=lk[:], in0=pre[:], scalar1=NEG_SLOPE,
                                    scalar2=None, op0=ALU.mult)
            nc.vector.tensor_tensor(out=lk[:], in0=lk[:], in1=pre[:], op=ALU.max)
            w = ep.tile([P, TC], DT.bfloat16, tag="w")
            nc.scalar.activation(w[:], lk[:], AF.Exp)

            M = mwp.tile([P, G, TC], DT.bfloat16, tag="M")
            nc.vector.tensor_tensor(
                out=M[:], in0=dstl[:, None, :].broadcast_to([P, G, TC]),
                in1=iota[:], op=ALU.is_equal)
            Mw = mwp.tile([P, G, TC], DT.bfloat16, tag="Mw")
            nc.vector.tensor_tensor(
                out=Mw[:], in0=M[:], in1=w[:, None, :].broadcast_to([P, G, TC]),
                op=ALU.mult)

            if pend is not None:
                _evac(pend)
            ps_list = []
            for k in range(PPC):
                ps = pp.tile([P, CS], DT.float32, tag="ps")
                for j in range(4):
                    gl = k * 4 + j
                    tt = gl * TPG
                    for t in range(TPG):
                        nc.tensor.matmul(ps[G * j:G * (j + 1), :],
                                         lhsT=Mw[:, :, tt + t],
                                         rhs=S[:, tt + t, :],
                                         start=(t == 0), stop=(t == TPG - 1),
                                         tile_position=(0, G * j))
                ps_list.append(ps)
            pend = (ps_list, ci)
        _evac(pend)
    nc.compile()
    return nc


def _get(key, builder, *a):
    if key not in _CACHE:
        _CACHE[key] = builder(*a)
    return _CACHE[key]


# ------------------------------------------------------------------ host ----

def _bin_pack(deg):
    """LPT: assign NSH nodes to NGRP bins of exactly G slots, load<=TPG*P.
    Returns perm [NGRP*G] int32 (node id or -1 for pad)."""
    import heapq
    order = np.argsort(-deg, kind="stable")
    heap = [(0, g) for g in range(NGRP)]
    heapq.heapify(heap)
    bins = [[] for _ in range(NGRP)]
    spill = []
    for n in order:
        d = int(deg[n])
        load, g = heapq.heappop(heap)
        while len(bins[g]) >= G:
            load, g = heapq.heappop(heap)
        bins[g].append(n)
        nl = load + d
        if nl > TPG * P:
            raise RuntimeError(f"bin overflow {nl}")
        if len(bins[g]) < G:
            heapq.heappush(heap, (nl, g))
        else:
            spill.append((nl, g))
    perm = np.full(NGRP * G, -1, dtype=np.int64)
    for g, lst in enumerate(bins):
        perm[g * G:g * G + len(lst)] = lst
    return perm


def _prep_graph(edge_index):
    """Per-core slot layout. Returns list of dicts."""
    ei = np.asarray(edge_index)
    src = np.concatenate([ei[0], np.arange(N, dtype=ei.dtype)]).astype(np.int64)
    dst = np.concatenate([ei[1], np.arange(N, dtype=ei.dtype)]).astype(np.int64)
    owner = dst // NSH
    cores = []
    for c in range(NCORES):
        sel = owner == c
        s_c = src[sel]
        d_c = dst[sel] - c * NSH                     # local dst 0..12499
        deg = np.bincount(d_c, minlength=NSH)
        perm = _bin_pack(deg)                        # [12544] node or -1
        # node -> (group, j)
        slot_of_node = np.full(NSH, -1, dtype=np.int64)
        valid = perm >= 0
        slot_of_node[perm[valid]] = np.nonzero(valid)[0]
        key = slot_of_node[d_c]                      # g*32+j per edge
        order = np.argsort(key, kind="stable")
        s_c, d_c, key = s_c[order], d_c[order], key[order]
        grp = key // G
        # position within group: running index
        gstart = np.searchsorted(grp, np.arange(NGRP))
        cnt = np.diff(np.append(gstart, len(grp)))
        if cnt.max() > TPG * P:
            raise RuntimeError(f"group overflow {cnt.max()}")
        pos = np.arange(len(grp)) - gstart[grp]
        slot = grp * (TPG * P) + pos                 # linear slot in [0, NSLOT)
        slot_src = np.zeros(NSLOT, dtype=np.int64)
        slot_dst_g = np.zeros(NSLOT, dtype=np.int64) # global dst per slot
        dstloc = np.zeros(NSLOT, dtype=np.float32)
        wkill = np.full(NSLOT, True)                 # pad slots
        slot_src[slot] = s_c
        slot_dst_g[slot] = d_c + c * NSH
        dstloc[slot] = key % G
        wkill[slot] = False
        cores.append(dict(slot_src=slot_src, slot_dst=slot_dst_g,
                          dstloc=dstloc.astype(BF16), wkill=wkill, perm=perm))
    return cores


def _make_iota():
    i = np.arange(G, dtype=np.float32)[None, :, None]
    return np.broadcast_to(i, (P, G, TC)).astype(BF16).copy()


def _feats_stream(table66, core):
    """table66 [N,65] (col 64 = 1.0). -> [P, NT, CS] stream dtype."""
    flat = table66[core["slot_src"]]                 # [NSLOT, 65]
    flat[core["wkill"], 64] = 0                      # ones col 0 on pad slots
    return np.ascontiguousarray(
        flat.reshape(NT, P, CS).transpose(1, 0, 2))


def _meta_streams(pre_f32, core):
    pre = pre_f32.copy()
    pre[core["wkill"]] = -30000.0
    pre = pre.astype(BF16).reshape(NT, P).T.copy()
    dstl = core["dstloc"].reshape(NT, P).T.copy()
    return pre, dstl


def _run_lin(nc_lin, xT_list, W, a_src, a_dst):
    Wb = np.ascontiguousarray(W, dtype=np.float32).astype(BF16)
    WTb = np.ascontiguousarray(W.T, dtype=np.float32).astype(BF16)
    ap = np.stack([a_src, a_dst], axis=1).astype(np.float32).astype(BF16)
    in_maps = [{"xT": xT_list[c], "w": Wb, "wT": WTb, "apair": ap}
               for c in range(NCORES)]
    res = run_bass_kernel_spmd(nc_lin, in_maps, core_ids=list(range(NCORES)))
    # assemble global tables: xs [N,64] bf16 (from cols 0:NSH), s,d [N] f32
    xs = np.empty((N, NHID + 2), dtype=np.float32)
    for c in range(NCORES):
        xs[c * NSH:(c + 1) * NSH] = \
            res.results[c]["xs_sd"][:, :NSH].T.astype(np.float32)
    return xs[:, 0:NHID], xs[:, NHID], xs[:, NHID + 1]


def _run_agg(nc_agg, cores, xs, s, d, iota, fp8):
    table66 = np.empty((N, CS), dtype=np.float32)
    table66[:, 0:NHID] = xs
    table66[:, NHID] = 1.0
    table66 = table66.astype(F8 if fp8 else BF16)
    in_maps = []
    for core in cores:
        pre = s[core["slot_src"]] + d[core["slot_dst"]]
        pre_st, dst_st = _meta_streams(pre, core)
        in_maps.append({"feats": _feats_stream(table66, core),
                        "pre": pre_st, "dstloc": dst_st, "iota": iota})
    res = run_bass_kernel_spmd(nc_agg, in_maps, core_ids=list(range(NCORES)))
    # out [P, NPS, 64] -> rows r = pstile*128+p = g*32+j -> node perm[g*32+j]
    full = np.empty((N, NHID), dtype=np.float32)
    for c, core in enumerate(cores):
        o = res.results[c]["out"]                   # [P, NPS, 64]
        rows = o.transpose(1, 0, 2).reshape(NODES_PAD, NHID).astype(np.float32)
        valid = core["perm"] >= 0
        full[c * NSH + core["perm"][valid]] = rows[valid]
    return full


def kernel(x, W1, att_src1, att_dst1, W2, att_src2, att_dst2, edge_index):
    x = np.asarray(x, dtype=np.float32)
    W1 = np.asarray(W1, dtype=np.float32)
    W2 = np.asarray(W2, dtype=np.float32)
    a_s1 = np.asarray(att_src1, dtype=np.float32)
    a_d1 = np.asarray(att_dst1, dtype=np.float32)
    a_s2 = np.asarray(att_src2, dtype=np.float32)
    a_d2 = np.asarray(att_dst2, dtype=np.float32)

    cores = _prep_graph(edge_index)
    iota = _make_iota()

    ncA = _get(("lin", NFEAT), _build_lin, NFEAT)
    ncB2 = _get(("lin", NHID), _build_lin, NHID)
    ncB = _get(("agg", True), _build_agg, True, False, FP8_L1)
    ncC = _get(("agg", False), _build_agg, False, True, False)

    # layer 1
    xb = x.astype(BF16)
    xT_list = []
    for c in range(NCORES):
        xt = np.zeros((NFEAT, NODES_PAD), dtype=BF16)
        xt[:, :NSH] = xb[c * NSH:(c + 1) * NSH].T
        xT_list.append(xt)
    xs1, s1, d1 = _run_lin(ncA, xT_list, W1, a_s1, a_d1)
    h = _run_agg(ncB, cores, xs1, s1, d1, iota, FP8_L1)

    # layer 2
    hb = h.astype(BF16)
    hT_list = []
    for c in range(NCORES):
        ht = np.zeros((NHID, NODES_PAD), dtype=BF16)
        ht[:, :NSH] = hb[c * NSH:(c + 1) * NSH].T
        hT_list.append(ht)
    xs2, s2, d2 = _run_lin(ncB2, hT_list, W2, a_s2, a_d2)
    out = _run_agg(ncC, cores, xs2, s2, d2, iota, False)
    return out.astype(np.float32)
